# revision 7
# baseline (speedup 1.0000x reference)
"""Causal self-attention (B=4, T=2048, C=1024, H=16) on 8 Trainium2 cores.

Sharding: core c = (batch b = c//2, head-half hg = c%2). Each core computes
q/k/v for its 8 heads over the full sequence of its batch, runs causal
attention, then the pair (b,0)/(b,1) exchanges attention outputs (AllGather)
and each core computes its half of the output columns of the final
projection. Host assembles out[b, :, hg*512:(hg+1)*512] from the cores.

All matmuls run as float32r (TF32) on the PE; softmax exp on the scalar
engine; accumulation in fp32 PSUM. No numerically-needed max-subtraction:
scores are ~N(0,1), so raw exp is safe.

Device layouts (partition dim first):
  xt  [128, 8, 2048]   x^T (C on partitions in 8 chunks; T free)
  qt  [128, 4, 2048]   q^T (head l -> chunk l//2, partitions 64*(l%2)..+64)
  kt  [128, 4, 2048]   k^T (same layout)
  vv  [128, 16, 520]   v natural (T on partitions, 16 chunks; head-major cols
                       with a ones column every 65th col -> softmax sums)
  yt  [64, 8, 2048]    normalized (att @ v)^T, head l -> chunk l
Attention inner loop (head l, q-chunk qc of 512, key-block kb of 128):
  S^T[k,q] = kt_l[:,kb].T @ qt_l[:,qc]          PSUM [128, <=512]
  att = exp(S^T/8)  (ACT, PSUM->SBUF); tri-mask the diagonal block (DVE)
  [y^T; sums] += [v_l | 1].T @ att              PSUM [65, <=512] over kb
  sums -> partition 0 (PE), recip (DVE), broadcast to 64 rows (PE),
  y^T * recip -> yt (DVE)
"""
import sys

sys.path.insert(0, "/opt/trn_rl_repo")

import numpy as np

import concourse.bacc as bacc
import concourse.bass as bass
import concourse.mybir as mybir
import concourse.tile as tile
from concourse import bass_utils

F32 = mybir.dt.float32
F32R = mybir.dt.float32r
AF = mybir.ActivationFunctionType

B, T, C, H, D = 4, 2048, 1024, 16, 64
HL = 8          # heads per core
CL = HL * D     # 512: per-core slice of C
NCORES = 8
QC = 512        # q-chunk width
NQC = T // QC   # 4
SCALE = 1.0 / np.sqrt(D)

_CACHE = {}


def _build():
    nc = bacc.Bacc("TRN2", target_bir_lowering=False, debug=False, num_devices=NCORES)

    x_d = nc.dram_tensor("x", [T, C], F32R, kind="ExternalInput")
    wq_d = nc.dram_tensor("wq", [C, CL], F32R, kind="ExternalInput")
    wk_d = nc.dram_tensor("wk", [C, CL], F32R, kind="ExternalInput")
    wv_d = nc.dram_tensor("wv", [C, CL], F32R, kind="ExternalInput")
    bq_d = nc.dram_tensor("bq", [1, CL], F32R, kind="ExternalInput")
    bk_d = nc.dram_tensor("bk", [1, CL], F32R, kind="ExternalInput")
    bv_d = nc.dram_tensor("bv", [1, CL], F32R, kind="ExternalInput")
    wp_d = nc.dram_tensor("wp", [C, CL], F32R, kind="ExternalInput")
    bp_d = nc.dram_tensor("bp", [1, CL], F32R, kind="ExternalInput")
    id_d = nc.dram_tensor("ident", [128, 128], F32R, kind="ExternalInput")
    tri_d = nc.dram_tensor("tri", [128, 128], F32R, kind="ExternalInput")
    ones_d = nc.dram_tensor("ones", [1, T], F32R, kind="ExternalInput")
    ones2_d = nc.dram_tensor("ones2", [128, 128], F32R, kind="ExternalInput")
    out_d = nc.dram_tensor("out", [T, CL], F32, kind="ExternalOutput")

    # collective bounce buffers (internal DRAM)
    yin_d = nc.dram_tensor("yin", [CL, T], F32R)
    yall_d = nc.dram_tensor("yall", [C, T], F32R)

    with tile.TileContext(nc) as tc:
        with (
            tc.tile_pool(name="const", bufs=1) as cpool,
            tc.tile_pool(name="big", bufs=1) as bpool,
        ):
            ident = cpool.tile([128, 128], F32R, tag="ident")
            tri = cpool.tile([128, 128], F32R, tag="tri")
            ones = cpool.tile([1, T], F32R, tag="ones")
            ones2 = cpool.tile([128, 128], F32R, tag="ones2")
            bq = cpool.tile([1, CL], F32R, tag="bq")
            bk = cpool.tile([1, CL], F32R, tag="bk")
            bv = cpool.tile([1, CL], F32R, tag="bv")
            bp = cpool.tile([1, CL], F32R, tag="bp")
            nc.sync.dma_start(ident[:], id_d[:])
            nc.sync.dma_start(tri[:], tri_d[:])
            nc.sync.dma_start(ones[:], ones_d[:])
            nc.sync.dma_start(bq[:], bq_d[:])
            nc.sync.dma_start(bk[:], bk_d[:])
            nc.sync.dma_start(bv[:], bv_d[:])
            nc.sync.dma_start(bp[:], bp_d[:])
            nc.sync.dma_start(ones2[:], ones2_d[:])

            qt = bpool.tile([128, 4, T], F32R, tag="qt")
            kt = bpool.tile([128, 4, T], F32R, tag="kt")
            vv = bpool.tile([128, 16, HL * (D + 1)], F32R, tag="vv")
            vview = vv[:].rearrange("p t (l e) -> p t l e", l=HL)

            # ---- Phase 0: x -> x^T;  Phase 1: q^T, k^T, v ---------------
            with (
                tc.tile_pool(name="p0", bufs=2) as p0,
                tc.tile_pool(name="p0ps", bufs=2, space=bass.MemorySpace.PSUM) as p0ps,
                tc.tile_pool(name="xtp", bufs=1) as xtp,
            ):
                xt = xtp.tile([128, 8, T], F32R, tag="xt")
                for ti in range(T // 128):
                    xs = p0.tile([128, C], F32R, tag="xs")
                    nc.sync.dma_start(xs[:], x_d[ti * 128 : (ti + 1) * 128, :])
                    for cg in range(2):  # 4 transposed chunks per PSUM bank
                        tps = p0ps.tile([128, 4, 128], F32R, tag="tp")
                        for j in range(4):
                            cc = cg * 4 + j
                            nc.tensor.transpose(
                                tps[:, j, :], xs[:, cc * 128 : (cc + 1) * 128], ident[:]
                            )
                        nc.vector.tensor_copy(
                            xt[:, cg * 4 : (cg + 1) * 4, ti * 128 : (ti + 1) * 128],
                            tps[:],
                        )

                with (
                    tc.tile_pool(name="p1w", bufs=1) as p1w,
                    tc.tile_pool(name="p1ps", bufs=3, space=bass.MemorySpace.PSUM) as p1ps,
                ):
                    nc.vector.tensor_copy(
                        vview[:, :, :, D : D + 1],
                        ones2[:].rearrange("p (t l e) -> p t l e", t=16, l=HL),
                    )

                    for w_d, b_sb, dst in [(wq_d, bq, qt), (wk_d, bk, kt)]:
                        w_sb = p1w.tile([128, 8, CL], F32R, tag="w")
                        nc.sync.dma_start(
                            w_sb[:], w_d.ap().rearrange("(c p) n -> p c n", p=128)
                        )
                        for m in range(4):  # 128-row chunk of head-dim rows
                            for t4 in range(4):  # T in 512-chunks
                                acc = p1ps.tile([128, QC], F32, tag="g")
                                for cc in range(8):
                                    nc.tensor.matmul(
                                        acc[:],
                                        w_sb[:, cc, m * 128 : (m + 1) * 128],
                                        xt[:, cc, t4 * QC : (t4 + 1) * QC],
                                        start=(cc == 0),
                                        stop=False,
                                    )
                                nc.tensor.matmul(
                                    acc[:],
                                    b_sb[:, m * 128 : (m + 1) * 128],
                                    ones[:, t4 * QC : (t4 + 1) * QC],
                                    start=False,
                                    stop=True,
                                )
                                nc.vector.tensor_copy(
                                    dst[:, m, t4 * QC : (t4 + 1) * QC], acc[:]
                                )

                    w_sb = p1w.tile([128, 8, CL], F32R, tag="w")
                    nc.sync.dma_start(
                        w_sb[:], wv_d.ap().rearrange("(c p) n -> p c n", p=128)
                    )
                    for ti in range(T // 128):
                        acc = p1ps.tile([128, CL], F32, tag="g")
                        for cc in range(8):
                            nc.tensor.matmul(
                                acc[:],
                                xt[:, cc, ti * 128 : (ti + 1) * 128],
                                w_sb[:, cc, :],
                                start=(cc == 0),
                                stop=False,
                            )
                        nc.tensor.matmul(
                            acc[:], ones[:, 0:128], bv[:], start=False, stop=True
                        )
                        nc.scalar.copy(
                            vview[:, ti, :, 0:D],
                            acc[:].rearrange("p (l e) -> p l e", l=HL),
                        )

            # ---- Phase 2: attention;  Phase 3: pair exchange ------------
            with (
                tc.tile_pool(name="ytp", bufs=1) as ytp,
                tc.tile_pool(name="p2", bufs=4) as p2,
                tc.tile_pool(name="p2s", bufs=3, space=bass.MemorySpace.PSUM) as p2s,
                tc.tile_pool(name="p2y", bufs=2, space=bass.MemorySpace.PSUM) as p2y,
                tc.tile_pool(name="p2b", bufs=2, space=bass.MemorySpace.PSUM) as p2b,
            ):
                yt = ytp.tile([64, HL, T], F32R, tag="yt")
                for l in range(HL):
                    pb = 64 * (l % 2)
                    ch = l // 2
                    for qc in range(NQC):
                        q0 = qc * QC
                        nkb = 4 * qc + 4
                        yp = p2y.tile([D + 1, QC], F32, tag="y")
                        for kb in range(nkb):
                            j = kb - 4 * qc  # >=0 only inside diagonal band
                            w0 = j * 128 if j > 0 else 0  # valid stripe start
                            sp = p2s.tile([128, QC], F32, tag="s")
                            nc.tensor.matmul(
                                sp[:, w0:QC],
                                kt[pb : pb + 64, ch, kb * 128 : (kb + 1) * 128],
                                qt[pb : pb + 64, ch, q0 + w0 : q0 + QC],
                                start=True,
                                stop=True,
                            )
                            att = p2.tile([128, QC], F32R, tag="att")
                            nc.scalar.activation(
                                att[:, w0:QC], sp[:, w0:QC], AF.Exp, scale=SCALE
                            )
                            if j >= 0:
                                nc.vector.tensor_mul(
                                    att[:, j * 128 : (j + 1) * 128],
                                    att[:, j * 128 : (j + 1) * 128],
                                    tri[:],
                                )
                            nc.tensor.matmul(
                                yp[:, w0:QC],
                                vv[:, kb, l * (D + 1) : (l + 1) * (D + 1)],
                                att[:, w0:QC],
                                start=(kb == 0),
                                stop=(kb == nkb - 1),
                            )
                        # normalize rows 0..63 by row 64 (the exp-sums)
                        rc = p2.tile([D + 1, QC], F32R, tag="rc")
                        with nc.allow_low_precision(reason="tf32 softmax recip"):
                            nc.vector.reciprocal(rc[D : D + 1, :], yp[D : D + 1, :])
                        bcp = p2b.tile([D, QC], F32, tag="bc")
                        nc.tensor.matmul(
                            bcp[:], ones2[64:65, 0:D], rc[D : D + 1, :],
                            start=True, stop=True,
                        )
                        bcs = p2.tile([D, QC], F32R, tag="bcs")
                        nc.scalar.copy(bcs[:], bcp[:])
                        nc.vector.tensor_mul(
                            yt[:, l, q0 : q0 + QC], yp[0:D, :], bcs[:]
                        )

                nc.sync.dma_start(
                    yin_d.ap().rearrange("(l p) t -> p l t", p=64), yt[:]
                )
                nc.gpsimd.collective_compute(
                    "AllGather",
                    mybir.AluOpType.bypass,
                    replica_groups=[[0, 1], [2, 3], [4, 5], [6, 7]],
                    ins=[yin_d.ap().opt()],
                    outs=[yall_d.ap().opt()],
                )

            # ---- Phase 4: projection (own 512 output columns) -----------
            with (
                tc.tile_pool(name="p4", bufs=3) as p4,
                tc.tile_pool(name="p4y", bufs=1) as p4y,
                tc.tile_pool(name="p4ps", bufs=3, space=bass.MemorySpace.PSUM) as p4ps,
            ):
                ya = p4y.tile([128, 8, T], F32R, tag="ya")
                nc.sync.dma_start(
                    ya[:], yall_d.ap().rearrange("(g p) t -> p g t", p=128)
                )
                wp_sb = p4y.tile([128, 8, CL], F32R, tag="wp")
                nc.sync.dma_start(
                    wp_sb[:], wp_d.ap().rearrange("(c p) n -> p c n", p=128)
                )
                for ti in range(T // 128):
                    acc = p4ps.tile([128, CL], F32, tag="p")
                    for g in range(8):
                        nc.tensor.matmul(
                            acc[:],
                            ya[:, g, ti * 128 : (ti + 1) * 128],
                            wp_sb[:, g, :],
                            start=(g == 0),
                            stop=False,
                        )
                    nc.tensor.matmul(
                        acc[:], ones[:, 0:128], bp[:], start=False, stop=True
                    )
                    o_sb = p4.tile([128, CL], F32, tag="o")
                    nc.vector.tensor_copy(o_sb[:], acc[:])
                    nc.sync.dma_start(out_d[ti * 128 : (ti + 1) * 128, :], o_sb[:])

    nc.compile()
    return nc


def kernel(x, W_attn, b_attn, W_proj, b_proj):
    x = np.asarray(x, dtype=np.float32)
    W_attn = np.asarray(W_attn, dtype=np.float32)
    b_attn = np.asarray(b_attn, dtype=np.float32)
    W_proj = np.asarray(W_proj, dtype=np.float32)
    b_proj = np.asarray(b_proj, dtype=np.float32)

    if "nc" not in _CACHE:
        _CACHE["nc"] = _build()
    nc = _CACHE["nc"]

    ident = np.eye(128, dtype=np.float32)
    tri = np.triu(np.ones((128, 128), dtype=np.float32))
    ones = np.ones((1, T), dtype=np.float32)

    in_maps = []
    for c in range(NCORES):
        b, hg = c // 2, c % 2
        cs = hg * CL
        in_maps.append(
            {
                "x": np.ascontiguousarray(x[b]),
                "wq": np.ascontiguousarray(W_attn[:, cs : cs + CL]),
                "wk": np.ascontiguousarray(W_attn[:, C + cs : C + cs + CL]),
                "wv": np.ascontiguousarray(W_attn[:, 2 * C + cs : 2 * C + cs + CL]),
                "bq": np.ascontiguousarray(b_attn[None, cs : cs + CL]),
                "bk": np.ascontiguousarray(b_attn[None, C + cs : C + cs + CL]),
                "bv": np.ascontiguousarray(b_attn[None, 2 * C + cs : 2 * C + cs + CL]),
                "wp": np.ascontiguousarray(W_proj[:, cs : cs + CL]),
                "bp": np.ascontiguousarray(b_proj[None, cs : cs + CL]),
                "ident": ident,
                "tri": tri,
                "ones": ones,
                "ones2": np.ones((128, 128), dtype=np.float32),
            }
        )

    res = bass_utils.run_bass_kernel_spmd(nc, in_maps, core_ids=list(range(NCORES)))

    out = np.empty((B, T, C), dtype=np.float32)
    for c in range(NCORES):
        b, hg = c // 2, c % 2
        out[b, :, hg * CL : (hg + 1) * CL] = res.results[c]["out"]
    return out
